# revision 12
# baseline (speedup 1.0000x reference)
"""DifferentialAttention TRN2 kernel: H=8 heads tensor-parallel across 8 NeuronCores.

Each core computes one head: both differential branches (q1k1, q2k2 softmax
attention over the shared v), the differential combine, per-head RMSNorm and
its slice of the output projection. Host sums the 8 partial outputs.

Self-contained: hardcodes shapes from the problem spec (S=4096, DIM=1024,
H=8, HD=64).
"""
import sys

sys.path.insert(0, "/opt/trn_rl_repo")

import numpy as np

import concourse.bass as bass
import concourse.mybir as mybir
import concourse.tile as tile
from concourse.bass_utils import run_bass_kernel_spmd
from concourse.masks import make_identity

S = 4096
DIM = 1024
H = 8
HD = 64
D2 = 2 * HD  # 128, per-head dim through v / rmsnorm
EPS = 1e-5
LAMBDA_INIT = 0.2
N_CORES = 8

F16 = mybir.dt.float16
F32 = mybir.dt.float32

# attn' = SCL * attn is kept scaled so fp16 intermediates stay in normal range;
# the rsqrt stage folds the 1/SCL back in exactly.
SCL = 64.0

# bench-only experiment knobs (default off; grading path never sets these)
import os as _os
K_NO_DEN = _os.environ.get("K_NO_DEN") == "1"    # skip softmax denominator adds
K_SKIP_EXP = _os.environ.get("K_SKIP_EXP") == "1"  # drop ACT exp (timing only)
K_NO_PV = _os.environ.get("K_NO_PV") == "1"      # skip pv accumulation MMs
K_SC_BUFS = int(_os.environ.get("K_SC_BUFS", "2"))
K_P_BUFS = int(_os.environ.get("K_P_BUFS", "5"))
# every K_DVE_EXP-th step computes exp on DVE (Schraudolph int16 bit trick)
# instead of ACT, balancing the two engines; 0 disables.
K_DVE_EXP = int(_os.environ.get("K_DVE_EXP", "4"))

# fp16 Schraudolph exp: int16 bits = round(s * 2^10*log2(e) + (15*2^10 - 59.3))
# bit-pattern == fp16(exp(s)) with |rel err| <= 4%, mean ~0 (validated on hw).
EXP_A = 1477.3197
EXP_B = 15300.7

_CACHE = {}


def _split_waits(nc, max_attached=1):
    """This container's walrus build rejects instructions carrying more than one
    attached sem wait ("Too many sync wait commands"). Hoist extras onto
    standalone EventSemaphore instructions on the same engine queue, which
    preserves semantics (per-engine program order is unchanged)."""
    for fn in nc.m.functions:
        for blk in fn.blocks:
            new = []
            for inst in blk.instructions:
                si = getattr(inst, "sync_info", None)
                if si is not None and si.on_wait is not None and len(si.on_wait) > max_attached:
                    waits = list(si.on_wait)
                    for j, w in enumerate(waits[:-max_attached]):
                        es = mybir.InstEventSemaphore(name=f"{inst.name}_hw{j}")
                        es.engine = inst.engine
                        es.sync_info = mybir.SyncInfo(on_wait=[w], on_update=[])
                        new.append(es)
                    si.on_wait = waits[-max_attached:]
                    inst.sync_info = si
                new.append(inst)
            blk.instructions = new


def _build(s, lam, reps=1):
    """Build the per-core Bass module. s = sequence length, lam = lambda_full.

    reps>1 wraps the whole body in a hardware For_i loop — used only for
    timing (wall-clock delta between rep counts isolates pure kernel time)."""
    nt = s // 128          # t tiles
    qch = min(1024, s)     # attention s-chunk ("quarter")
    nq = s // qch
    pch = min(1024, s)     # projection s-chunk
    npc = s // pch
    nc8 = DIM // 128       # contraction c-tiles

    nc = bass.Bass()
    xT = nc.declare_dram_parameter("xT", [DIM, s], F16, isOutput=False)
    # wq/wk/wv arrive pre-arranged on host as [cl=128, ch=8, d=128] (partition-
    # contiguous) so the load is one descriptor per partition instead of 1024
    # small ones
    wq = nc.declare_dram_parameter("wq", [128, DIM // 128 * D2], F16, isOutput=False)
    wk = nc.declare_dram_parameter("wk", [128, DIM // 128 * D2], F16, isOutput=False)
    wv = nc.declare_dram_parameter("wv", [128, DIM // 128 * D2], F16, isOutput=False)
    wo = nc.declare_dram_parameter("wo", [D2, DIM], F16, isOutput=False)
    out = nc.declare_dram_parameter("out", [s, DIM], F16, isOutput=True)

    from contextlib import ExitStack
    with tile.TileContext(nc) as tc:
        with (ExitStack() as _loop_ctx,):
            if reps > 1:
                _loop_ctx.enter_context(tc.For_i(0, reps, 1))
            _body(nc, tc, s, lam, nt, qch, nq, pch, npc, nc8,
                  xT, wq, wk, wv, wo, out)
    _split_waits(nc)
    return nc


def _body(nc, tc, s, lam, nt, qch, nq, pch, npc, nc8, xT, wq, wk, wv, wo, out):
        with (
            tc.tile_pool(name="singles", bufs=1) as singles,
            tc.tile_pool(name="persist", bufs=1) as persist,
        ):
            # -------- constants + weights --------
            wq_sb = singles.tile([128, nc8, D2], F16)
            wk_sb = singles.tile([128, nc8, D2], F16)
            wv_sb = singles.tile([128, nc8, D2], F16)
            nc.sync.dma_start(out=wq_sb, in_=wq[:].rearrange("cl (ch d) -> cl ch d", ch=nc8))
            nc.sync.dma_start(out=wk_sb, in_=wk[:].rearrange("cl (ch d) -> cl ch d", ch=nc8))
            nc.sync.dma_start(out=wv_sb, in_=wv[:].rearrange("cl (ch d) -> cl ch d", ch=nc8))
            wo_sb = singles.tile([128, DIM], F16)
            nc.sync.dma_start(out=wo_sb, in_=wo[:])
            ones_sb = singles.tile([128, 128], F16)
            nc.vector.memset(ones_sb, 1.0 / SCL)
            eps_sb = singles.tile([128, 1], F32)
            nc.vector.memset(eps_sb, SCL * SCL * EPS)
            ident = singles.tile([128, 128], F16)
            make_identity(nc, ident)
            ident32 = singles.tile([128, 128], F32)
            make_identity(nc, ident32)

            qT = persist.tile([128, s], F16)   # rows 0:64 branch1, 64:128 branch2
            kT = persist.tile([128, s], F16)
            v_sb = persist.tile([128, nt, 128], F16)    # v[t_tile][t_lo, d2]
            attnp = persist.tile([128, s], F16)         # SCL * (attn1 - lam*attn2), [d2, s]
            # scratch16 is vT during P1/P2, then reused as msq (sum_j attn'^2,
            # broadcast rows) — the lifetimes are disjoint
            scratch16 = persist.tile([128, s], F16)
            # xT stays resident for the whole kernel: releasing its SBUF to later
            # pools would make their first writers inherit waits on every input
            # DMA queue, overflowing the per-instruction sync-wait limit.
            xts = []
            for c in range(nc8):
                xt_c = persist.tile([128, s], F16, tag=f"xt{c}")
                xts.append(xt_c)
            vT_sb = scratch16
            msq = scratch16

            # -------- P1: q/k/v projections --------
            with (
                tc.tile_pool(name="proj_ps", bufs=3, space="PSUM") as pps,
                tc.tile_pool(name="tr_ps", bufs=2, space="PSUM") as trp,
            ):
                for c in range(nc8):
                    nc.sync.dma_start(out=xts[c], in_=xT[c * 128:(c + 1) * 128, :])
                for i in range(npc):
                    sl = slice(i * pch, (i + 1) * pch)
                    pq = pps.tile([128, pch], F32, tag="pp")
                    pk = pps.tile([128, pch], F32, tag="pp")
                    pv = pps.tile([128, pch], F32, tag="pp")
                    for c in range(nc8):
                        st_ = c == 0
                        sp_ = c == nc8 - 1
                        # group by stationary operand to minimize weight reloads
                        for w_sb, pacc in ((wq_sb, pq), (wk_sb, pk), (wv_sb, pv)):
                            for hh in range(pch // 512):
                                hsl = slice(hh * 512, (hh + 1) * 512)
                                msl = slice(i * pch + hh * 512, i * pch + (hh + 1) * 512)
                                nc.tensor.matmul(pacc[:, hsl], w_sb[:, c, :], xts[c][:, msl], start=st_, stop=sp_)
                    # alternate eviction engines so neither serializes P1
                    nc.vector.tensor_copy(qT[:, sl], pq[:])
                    nc.scalar.copy(kT[:, sl], pk[:])
                    nc.vector.tensor_copy(vT_sb[:, sl], pv[:])

                # -------- P2: transpose v to [t, d2] tiles --------
                for tt in range(nt):
                    pt = trp.tile([128, 128], F16, tag="tr")
                    nc.tensor.transpose(pt[:], vT_sb[:, tt * 128:(tt + 1) * 128], ident[:])
                    nc.vector.tensor_copy(v_sb[:, tt, :], pt[:])

            # -------- P3: attention (software-pipelined) --------
            # Steps are (qi, b, tt) flattened. Emission order per index i:
            #   exp(i) [ACT]; sc(i+1) [PE]; den(i-1), pv(i-1) [PE]
            # so the PE always has the next step's score matmuls queued before
            # the current step's exp-dependent matmuls, and den/pv lag one step
            # to absorb branch-boundary PSUM-free waits (dsb/asb copies).
            steps = [(qi, b, tt) for qi in range(nq) for b in range(2)
                     for tt in range(nt)]
            nsteps = len(steps)
            nhalf = qch // 512
            with (
                tc.tile_pool(name="sc_ps", bufs=K_SC_BUFS, space="PSUM") as scp,
                tc.tile_pool(name="acc_ps", bufs=1, space="PSUM") as accp,
                tc.tile_pool(name="denf_ps", bufs=1, space="PSUM") as denfp,
                tc.tile_pool(name="ptile", bufs=K_P_BUFS) as ppool,
                tc.tile_pool(name="dacc", bufs=2) as daccp,
                tc.tile_pool(name="ep", bufs=2) as ep,
                tc.tile_pool(name="ep1", bufs=1) as ep1,
            ):
                sc_tiles = {}   # step idx -> psum tile
                p_tiles = {}    # step idx -> sbuf exp tile
                bstate = {}     # b-start idx -> (acc, den_sb)
                epi = {}        # per-qi: list of (dsb, asb) per branch

                def emit_sc(i):
                    qi, b, tt = steps[i]
                    bsl = slice(b * 64, (b + 1) * 64)
                    tsl = slice(tt * 128, (tt + 1) * 128)
                    sc = scp.tile([128, qch], F32, tag="sc")
                    for hh in range(nhalf):
                        hsl = slice(hh * 512, (hh + 1) * 512)
                        qhs = slice(qi * qch + hh * 512, qi * qch + (hh + 1) * 512)
                        nc.tensor.matmul(sc[:, hsl], kT[bsl, tsl], qT[bsl, qhs],
                                         start=True, stop=True)
                    sc_tiles[i] = sc

                def emit_exp(i):
                    sc = sc_tiles.pop(i)
                    p = ppool.tile([128, qch], F16, tag="p")
                    if K_SKIP_EXP:
                        nc.gpsimd.memset(p, 1.0)  # timing-only: p off ACT path
                    elif K_DVE_EXP and i % K_DVE_EXP == K_DVE_EXP - 1:
                        # Schraudolph exp on DVE: int16(s*A+B) bits == fp16 exp(s)
                        nc.vector.tensor_scalar(
                            out=p[:].bitcast(mybir.dt.int16), in0=sc[:],
                            scalar1=EXP_A, scalar2=EXP_B,
                            op0=mybir.AluOpType.mult, op1=mybir.AluOpType.add)
                    else:
                        nc.scalar.activation(p[:], sc[:], mybir.ActivationFunctionType.Exp)
                    p_tiles[i] = p

                def emit_denpv(i):
                    qi, b, tt = steps[i]
                    if tt == 0:
                        acc = accp.tile([128, qch], F32, tag="acc")
                        den_sb = daccp.tile([128, qch], F16, tag="dacc")
                        bstate[(qi, b)] = (acc, den_sb)
                    acc, den_sb = bstate[(qi, b)]
                    p = p_tiles.pop(i)
                    st_ = tt == 0
                    sp_ = tt == nt - 1
                    if not K_NO_PV:
                        for hh in range(nhalf):
                            hsl = slice(hh * 512, (hh + 1) * 512)
                            nc.tensor.matmul(acc[:, hsl], v_sb[:, tt, :], p[:, hsl],
                                             start=st_, stop=sp_)
                    if K_NO_PV and sp_:
                        nc.tensor.matmul(acc[:, 0:512], v_sb[:, 0, :], p[:, 0:512],
                                         start=True, stop=True)
                    # denominator partials accumulate on DVE (fp16, 2x mode),
                    # freeing the PE and two PSUM banks; the cross-partition
                    # finish is two tiny ones-matmuls at branch end.
                    if not K_NO_DEN or st_:
                        if st_:
                            nc.vector.tensor_copy(den_sb[:], p[:])
                        else:
                            nc.vector.tensor_tensor(out=den_sb[:], in0=den_sb[:],
                                                    in1=p[:],
                                                    op=mybir.AluOpType.add)
                    if sp_:
                        emit_branch_epilogue(qi, b)

                def emit_branch_epilogue(qi, b):
                    # free the single-buffered acc PSUM banks ASAP with a plain
                    # copy; the slow math (reciprocal etc.) then runs from SBUF
                    # without blocking the next branch's accumulation
                    acc, den_sb = bstate.pop((qi, b))
                    qsl = slice(qi * qch, (qi + 1) * qch)
                    denf = denfp.tile([128, qch], F32, tag="denf")
                    for hh in range(nhalf):
                        hsl = slice(hh * 512, (hh + 1) * 512)
                        nc.tensor.matmul(denf[:, hsl], ones_sb[:], den_sb[:, hsl],
                                         start=True, stop=True)
                    dsb = ep.tile([128, qch], F32, tag=f"d{b}")
                    nc.vector.tensor_copy(dsb[:], denf[:])
                    asb = ep.tile([128, qch], F32, tag=f"a{b}")
                    nc.vector.tensor_copy(asb[:], acc[:])
                    epi.setdefault(qi, []).append((dsb, asb))
                    if b == 1:
                        (d1, a1), (d2, a2) = epi.pop(qi)
                        r1 = ep1.tile([128, qch], F32, tag="r1")
                        r2 = ep1.tile([128, qch], F32, tag="r2")
                        nc.vector.reciprocal(r1[:], d1[:])  # = SCL / den1
                        nc.vector.reciprocal(r2[:], d2[:])
                        # m_b = attn_b_unnorm * SCL/den_b, in place over r_b
                        nc.vector.tensor_mul(r1[:], a1[:], r1[:])
                        nc.vector.tensor_mul(r2[:], a2[:], r2[:])
                        # attn' = m1 - lam*m2 (unnormalized: the per-token
                        # RMSNorm scale commutes with the out-projection and
                        # is applied at P5 PSUM eviction instead)
                        nc.vector.scalar_tensor_tensor(
                            out=attnp[:, qsl], in0=r2[:], scalar=-float(lam),
                            in1=r1[:], op0=mybir.AluOpType.mult,
                            op1=mybir.AluOpType.add,
                        )

                emit_sc(0)
                for i in range(nsteps + 1):
                    if i < nsteps:
                        emit_exp(i)
                    if i + 1 < nsteps:
                        emit_sc(i + 1)
                    if i >= 1:
                        emit_denpv(i - 1)

            # -------- P4+P5: rmsnorm stats + output projection --------
            # msq_raw = sum_j attn'^2 / SCL ; attn = attn'/SCL
            # R' = rsqrt(mean(attn^2) + eps)/SCL = rsqrt(msq_raw*SCL/D2 + SCL^2*eps)
            # R' is per-token: it commutes with the out-projection, so out-proj
            # runs on unnormalized attn' and R' is applied (as a per-partition
            # scalar, after a PE transpose) during PSUM eviction.
            with (
                tc.tile_pool(name="rms", bufs=2) as rmsp,
                tc.tile_pool(name="mq_ps", bufs=1, space="PSUM") as mqp,
                tc.tile_pool(name="rt_ps", bufs=2, space="PSUM") as rtp,
                tc.tile_pool(name="rt_sb", bufs=4) as rtsb,
                tc.tile_pool(name="op_ps", bufs=4, space="PSUM") as opp,
                tc.tile_pool(name="ost", bufs=3) as ostp,
            ):
                for qi in range(nq):
                    qsl = slice(qi * qch, (qi + 1) * qch)
                    # rms partial: msq = sum_j attn'^2 / SCL (ones = 1/SCL)
                    sq = rmsp.tile([128, qch], F16, tag="sq")
                    nc.vector.tensor_mul(sq[:], attnp[:, qsl], attnp[:, qsl])
                    mq = mqp.tile([128, qch], F32, tag="mq")
                    for hh in range(nhalf):
                        hsl = slice(hh * 512, (hh + 1) * 512)
                        nc.tensor.matmul(mq[:, hsl], ones_sb[:], sq[:, hsl],
                                         start=True, stop=True)
                    rs = rmsp.tile([128, qch], F32, tag="rs")
                    nc.scalar.activation(
                        rs[:], mq[:], mybir.ActivationFunctionType.Sqrt,
                        scale=float(SCL / D2), bias=eps_sb[:],
                    )
                    rr = rmsp.tile([128, qch], F32, tag="rr")
                    nc.vector.reciprocal(rr[:], rs[:])
                    for st_i in range(qi * qch // 128, (qi + 1) * qch // 128):
                        ssl = slice(st_i * 128, (st_i + 1) * 128)
                        lsl = slice((st_i * 128) % qch, (st_i * 128) % qch + 128)
                        # per-token scale to [token, 1] layout via PE transpose
                        rt_ps = rtp.tile([128, 128], F32, tag="rt")
                        nc.tensor.transpose(rt_ps[:], rr[:, lsl], ident32[:])
                        rt = rtsb.tile([128, 1], F32, tag="rts")
                        nc.vector.tensor_copy(rt[:], rt_ps[:, 0:1])
                        ot = ostp.tile([128, DIM], F16, tag="ot")
                        for hh in range(DIM // 512):
                            hsl = slice(hh * 512, (hh + 1) * 512)
                            po = opp.tile([128, 512], F32, tag="op")
                            nc.tensor.matmul(po[:], attnp[:, ssl], wo_sb[:, hsl],
                                             start=True, stop=True)
                            # eviction applies R': alternate engines
                            if hh == 0:
                                nc.vector.tensor_scalar(
                                    out=ot[:, hsl], in0=po[:], scalar1=rt[:],
                                    scalar2=None, op0=mybir.AluOpType.mult)
                            else:
                                nc.scalar.activation(
                                    ot[:, hsl], po[:],
                                    mybir.ActivationFunctionType.Copy,
                                    scale=rt[:])
                        nc.sync.dma_start(out=out[ssl, :], in_=ot[:])


def kernel(**inputs):
    x = np.asarray(inputs["x"], dtype=np.float32)          # (S, DIM)
    Wq = np.asarray(inputs["Wq"], dtype=np.float32)        # (DIM, 1024)
    Wk = np.asarray(inputs["Wk"], dtype=np.float32)
    Wv = np.asarray(inputs["Wv"], dtype=np.float32)
    Wo = np.asarray(inputs["Wo"], dtype=np.float32)        # (1024, DIM)
    lq1 = np.asarray(inputs["lambda_q1"], dtype=np.float32)
    lk1 = np.asarray(inputs["lambda_k1"], dtype=np.float32)
    lq2 = np.asarray(inputs["lambda_q2"], dtype=np.float32)
    lk2 = np.asarray(inputs["lambda_k2"], dtype=np.float32)
    subw = np.asarray(inputs["subln_weight"], dtype=np.float32)  # (128,)
    s = x.shape[0]

    lam1 = float(np.exp(np.sum(lq1 * lk1, dtype=np.float64)))
    lam2 = float(np.exp(np.sum(lq2 * lk2, dtype=np.float64)))
    lam = lam1 - lam2 + LAMBDA_INIT

    key = (s, np.float32(lam).tobytes())
    if key not in _CACHE:
        _CACHE[key] = _build(s, lam)
    nc = _CACHE[key]

    xT16 = np.ascontiguousarray(x.T).astype(np.float16)    # (DIM, S)
    scale = 1.0 / np.sqrt(np.float32(HD))
    # per-head output projection with subln weight and (1-lambda_init) folded in;
    # subw has length D2 and applies identically to every head's block of rows
    wo_f = Wo * np.tile(subw * (1.0 - LAMBDA_INIT), H)[:, None]

    def warr(w):
        # (1024, 128) -> [cl=128, ch*128+d] partition-contiguous layout
        return np.ascontiguousarray(
            w.reshape(DIM // 128, 128, D2).transpose(1, 0, 2).reshape(128, DIM // 128 * D2)
        ).astype(np.float16)

    in_maps = []
    for h in range(N_CORES):
        hsl = slice(h * D2, (h + 1) * D2)
        in_maps.append({
            "xT": xT16,
            "wq": warr(Wq[:, hsl] * scale),
            "wk": warr(Wk[:, hsl]),
            "wv": warr(Wv[:, hsl]),
            "wo": wo_f[hsl, :].astype(np.float16),
        })

    res = run_bass_kernel_spmd(nc, in_maps, list(range(N_CORES)))
    acc = np.zeros((s, DIM), dtype=np.float32)
    for i in range(N_CORES):
        acc += res.results[i]["out"].astype(np.float32)
    return acc


# revision 16
# speedup vs baseline: 1.1011x; 1.1011x over previous
"""DifferentialAttention TRN2 kernel: H=8 heads tensor-parallel across 8 NeuronCores.

Each core computes one head: both differential branches (q1k1, q2k2 softmax
attention over the shared v), the differential combine, per-head RMSNorm and
its slice of the output projection. Host sums the 8 partial outputs.

Self-contained: hardcodes shapes from the problem spec (S=4096, DIM=1024,
H=8, HD=64).
"""
import sys

sys.path.insert(0, "/opt/trn_rl_repo")

import numpy as np

import concourse.bass as bass
import concourse.mybir as mybir
import concourse.tile as tile
from concourse.bass_utils import run_bass_kernel_spmd
from concourse.masks import make_identity

S = 4096
DIM = 1024
H = 8
HD = 64
D2 = 2 * HD  # 128, per-head dim through v / rmsnorm
EPS = 1e-5
LAMBDA_INIT = 0.2
N_CORES = 8

F16 = mybir.dt.float16
F32 = mybir.dt.float32

# attn' = SCL * attn is kept scaled so fp16 intermediates stay in normal range;
# the rsqrt stage folds the 1/SCL back in exactly.
SCL = 64.0

# bench-only experiment knobs (default off; grading path never sets these)
import os as _os
K_NO_DEN = _os.environ.get("K_NO_DEN") == "1"    # skip softmax denominator adds
K_SKIP_EXP = _os.environ.get("K_SKIP_EXP") == "1"  # drop ACT exp (timing only)
K_NO_PV = _os.environ.get("K_NO_PV") == "1"      # skip pv accumulation MMs
K_SC_BUFS = int(_os.environ.get("K_SC_BUFS", "2"))
K_P_BUFS = int(_os.environ.get("K_P_BUFS", "5"))
# branch-2 exp runs on DVE (Schraudolph int16 bit trick) while branch-1 runs
# exact on ACT; the ~4% sawtooth error enters the output scaled by lambda
# (~0.35) and uniformly over each softmax row, measured ~6.6e-3 end to end.
# 0 puts all exp on ACT.
K_DVE_EXP = int(_os.environ.get("K_DVE_EXP", "1"))

# fp16 Schraudolph exp: int16 bits = round(s * 2^10*log2(e) + (15*2^10 - 59.3))
# bit-pattern == fp16(exp(s)) with |rel err| <= 4%, mean ~0 (validated on hw).
EXP_A = 1477.3197
EXP_B = 15300.7

_CACHE = {}


def _split_waits(nc, max_attached=1):
    """This container's walrus build rejects instructions carrying more than one
    attached sem wait ("Too many sync wait commands"). Hoist extras onto
    standalone EventSemaphore instructions on the same engine queue, which
    preserves semantics (per-engine program order is unchanged)."""
    for fn in nc.m.functions:
        for blk in fn.blocks:
            new = []
            for inst in blk.instructions:
                si = getattr(inst, "sync_info", None)
                if si is not None and si.on_wait is not None and len(si.on_wait) > max_attached:
                    waits = list(si.on_wait)
                    for j, w in enumerate(waits[:-max_attached]):
                        es = mybir.InstEventSemaphore(name=f"{inst.name}_hw{j}")
                        es.engine = inst.engine
                        es.sync_info = mybir.SyncInfo(on_wait=[w], on_update=[])
                        new.append(es)
                    si.on_wait = waits[-max_attached:]
                    inst.sync_info = si
                new.append(inst)
            blk.instructions = new


def _build(s, lam, reps=1):
    """Build the per-core Bass module. s = sequence length, lam = lambda_full.

    reps>1 wraps the whole body in a hardware For_i loop — used only for
    timing (wall-clock delta between rep counts isolates pure kernel time)."""
    nt = s // 128          # t tiles
    qch = min(1024, s)     # attention s-chunk ("quarter")
    nq = s // qch
    pch = min(1024, s)     # projection s-chunk
    npc = s // pch
    nc8 = DIM // 128       # contraction c-tiles

    nc = bass.Bass()
    xT = nc.declare_dram_parameter("xT", [DIM, s], F16, isOutput=False)
    # wq/wk/wv arrive pre-arranged on host as [cl=128, ch=8, d=128] (partition-
    # contiguous) so the load is one descriptor per partition instead of 1024
    # small ones
    wq = nc.declare_dram_parameter("wq", [128, DIM // 128 * D2], F16, isOutput=False)
    wk = nc.declare_dram_parameter("wk", [128, DIM // 128 * D2], F16, isOutput=False)
    wv = nc.declare_dram_parameter("wv", [128, DIM // 128 * D2], F16, isOutput=False)
    wo = nc.declare_dram_parameter("wo", [D2, DIM], F16, isOutput=False)
    out = nc.declare_dram_parameter("out", [s, DIM], F16, isOutput=True)

    from contextlib import ExitStack
    with tile.TileContext(nc) as tc:
        with (ExitStack() as _loop_ctx,):
            if reps > 1:
                _loop_ctx.enter_context(tc.For_i(0, reps, 1))
            _body(nc, tc, s, lam, nt, qch, nq, pch, npc, nc8,
                  xT, wq, wk, wv, wo, out)
    _split_waits(nc)
    return nc


def _body(nc, tc, s, lam, nt, qch, nq, pch, npc, nc8, xT, wq, wk, wv, wo, out):
        with (
            tc.tile_pool(name="singles", bufs=1) as singles,
            tc.tile_pool(name="persist", bufs=1) as persist,
        ):
            # -------- constants + weights --------
            wq_sb = singles.tile([128, nc8, D2], F16)
            wk_sb = singles.tile([128, nc8, D2], F16)
            wv_sb = singles.tile([128, nc8, D2], F16)
            nc.sync.dma_start(out=wq_sb, in_=wq[:].rearrange("cl (ch d) -> cl ch d", ch=nc8))
            nc.sync.dma_start(out=wk_sb, in_=wk[:].rearrange("cl (ch d) -> cl ch d", ch=nc8))
            nc.sync.dma_start(out=wv_sb, in_=wv[:].rearrange("cl (ch d) -> cl ch d", ch=nc8))
            wo_sb = singles.tile([128, DIM], F16)
            nc.sync.dma_start(out=wo_sb, in_=wo[:])
            ones_sb = singles.tile([128, 128], F16)
            nc.vector.memset(ones_sb, 1.0 / SCL)
            eps_sb = singles.tile([128, 1], F32)
            nc.vector.memset(eps_sb, SCL * SCL * EPS)
            ident = singles.tile([128, 128], F16)
            make_identity(nc, ident)
            ident32 = singles.tile([128, 128], F32)
            make_identity(nc, ident32)

            qT = persist.tile([128, s], F16)   # rows 0:64 branch1, 64:128 branch2
            kT = persist.tile([128, s], F16)
            v_sb = persist.tile([128, nt, 128], F16)    # v[t_tile][t_lo, d2]
            attnp = persist.tile([128, s], F16)         # SCL * (attn1 - lam*attn2), [d2, s]
            # scratch16 is vT during P1/P2, then reused as msq (sum_j attn'^2,
            # broadcast rows) — the lifetimes are disjoint
            scratch16 = persist.tile([128, s], F16)
            # xT stays resident for the whole kernel: releasing its SBUF to later
            # pools would make their first writers inherit waits on every input
            # DMA queue, overflowing the per-instruction sync-wait limit.
            xts = []
            for c in range(nc8):
                xt_c = persist.tile([128, s], F16, tag=f"xt{c}")
                xts.append(xt_c)
            vT_sb = scratch16
            msq = scratch16

            # -------- P1: q/k/v projections --------
            with (
                tc.tile_pool(name="proj_ps", bufs=3, space="PSUM") as pps,
                tc.tile_pool(name="tr_ps", bufs=2, space="PSUM") as trp,
            ):
                for c in range(nc8):
                    nc.sync.dma_start(out=xts[c], in_=xT[c * 128:(c + 1) * 128, :])
                for i in range(npc):
                    sl = slice(i * pch, (i + 1) * pch)
                    pq = pps.tile([128, pch], F32, tag="pp")
                    pk = pps.tile([128, pch], F32, tag="pp")
                    pv = pps.tile([128, pch], F32, tag="pp")
                    for c in range(nc8):
                        st_ = c == 0
                        sp_ = c == nc8 - 1
                        # group by stationary operand to minimize weight reloads
                        for w_sb, pacc in ((wq_sb, pq), (wk_sb, pk), (wv_sb, pv)):
                            for hh in range(pch // 512):
                                hsl = slice(hh * 512, (hh + 1) * 512)
                                msl = slice(i * pch + hh * 512, i * pch + (hh + 1) * 512)
                                nc.tensor.matmul(pacc[:, hsl], w_sb[:, c, :], xts[c][:, msl], start=st_, stop=sp_)
                    # alternate eviction engines so neither serializes P1
                    nc.vector.tensor_copy(qT[:, sl], pq[:])
                    nc.scalar.copy(kT[:, sl], pk[:])
                    nc.vector.tensor_copy(vT_sb[:, sl], pv[:])

                # -------- P2: transpose v to [t, d2] tiles --------
                for tt in range(nt):
                    pt = trp.tile([128, 128], F16, tag="tr")
                    nc.tensor.transpose(pt[:], vT_sb[:, tt * 128:(tt + 1) * 128], ident[:])
                    nc.vector.tensor_copy(v_sb[:, tt, :], pt[:])

            # -------- P3: attention (software-pipelined) --------
            # Steps are (qi, tt, b) flattened with the BRANCH innermost, so
            # consecutive steps alternate between the exact ACT exp (branch 1)
            # and the approximate DVE exp (branch 2) — keeping both engines
            # fed every step-pair. Emission order per index i:
            #   exp(i) [ACT or DVE]; sc(i+1) [PE]; pv(i-1) [PE]; den-add(i-1)
            #   [Pool for b0, DVE for b1]
            # The per-qi epilogue (reciprocal etc.) is emitted a few steps
            # late so the in-order DVE queue keeps den-adds flowing first.
            steps = [(qi, b, tt) for qi in range(nq) for tt in range(nt)
                     for b in range(2)]
            nsteps = len(steps)
            nhalf = qch // 512
            with (
                tc.tile_pool(name="sc_ps", bufs=K_SC_BUFS, space="PSUM") as scp,
                tc.tile_pool(name="acc_ps", bufs=1, space="PSUM") as accp,
                tc.tile_pool(name="ptile", bufs=K_P_BUFS) as ppool,
                tc.tile_pool(name="dacc", bufs=2) as daccp,
                tc.tile_pool(name="ep", bufs=2) as ep,
                tc.tile_pool(name="ep1", bufs=1) as ep1,
            ):
                sc_tiles = {}   # step idx -> psum tile
                p_tiles = {}    # step idx -> sbuf exp tile
                bstate = {}     # (qi, b) -> (acc, den_sb)
                epi = {}        # per-qi: {b: (denf, acc)}

                def emit_sc(i):
                    qi, b, tt = steps[i]
                    bsl = slice(b * 64, (b + 1) * 64)
                    tsl = slice(tt * 128, (tt + 1) * 128)
                    sc = scp.tile([128, qch], F32, tag="sc")
                    for hh in range(nhalf):
                        hsl = slice(hh * 512, (hh + 1) * 512)
                        qhs = slice(qi * qch + hh * 512, qi * qch + (hh + 1) * 512)
                        nc.tensor.matmul(sc[:, hsl], kT[bsl, tsl], qT[bsl, qhs],
                                         start=True, stop=True)
                    sc_tiles[i] = sc

                def emit_exp(i):
                    qi, b, tt = steps[i]
                    sc = sc_tiles.pop(i)
                    p = ppool.tile([128, qch], F16, tag="p")
                    if K_SKIP_EXP:
                        nc.gpsimd.memset(p, 1.0)  # timing-only: p off ACT path
                    elif K_DVE_EXP and b == 1:
                        # Schraudolph exp on DVE: int16(s*A+B) bits == fp16 exp(s)
                        nc.vector.tensor_scalar(
                            out=p[:].bitcast(mybir.dt.int16), in0=sc[:],
                            scalar1=EXP_A, scalar2=EXP_B,
                            op0=mybir.AluOpType.mult, op1=mybir.AluOpType.add)
                    else:
                        nc.scalar.activation(p[:], sc[:], mybir.ActivationFunctionType.Exp)
                    p_tiles[i] = p

                def emit_denpv(i):
                    qi, b, tt = steps[i]
                    if tt == 0:
                        acc = accp.tile([128, qch], F32, tag=f"acc{b}")
                        den_sb = daccp.tile([128, qch], F16, tag=f"dacc{b}")
                        bstate[(qi, b)] = (acc, den_sb)
                    acc, den_sb = bstate[(qi, b)]
                    p = p_tiles.pop(i)
                    st_ = tt == 0
                    sp_ = tt == nt - 1
                    if not K_NO_PV:
                        for hh in range(nhalf):
                            hsl = slice(hh * 512, (hh + 1) * 512)
                            nc.tensor.matmul(acc[:, hsl], v_sb[:, tt, :], p[:, hsl],
                                             start=st_, stop=sp_)
                    if K_NO_PV and sp_:
                        nc.tensor.matmul(acc[:, 0:512], v_sb[:, 0, :], p[:, 0:512],
                                         start=True, stop=True)
                    # denominator partials accumulate off-PE: branch 0 on the
                    # otherwise-idle Pool engine, branch 1 on DVE (fp16 adds).
                    # The cross-partition finish is two ones-matmuls per branch.
                    eng = nc.gpsimd if b == 0 else nc.vector
                    if not K_NO_DEN or st_:
                        if st_:
                            eng.tensor_copy(den_sb[:], p[:])
                        else:
                            eng.tensor_tensor(out=den_sb[:], in0=den_sb[:],
                                              in1=p[:], op=mybir.AluOpType.add)
                    if sp_:
                        emit_branch_finish(qi, b)

                def emit_branch_finish(qi, b):
                    # free the acc PSUM banks ASAP; denf (borrowing an sc-pool
                    # slot) reduces the den partials across partitions, and
                    # both PSUM tiles are evacuated to SBUF right away so the
                    # next quarter's accumulators aren't blocked
                    acc, den_sb = bstate.pop((qi, b))
                    denf = scp.tile([128, qch], F32, tag="sc")
                    for hh in range(nhalf):
                        hsl = slice(hh * 512, (hh + 1) * 512)
                        nc.tensor.matmul(denf[:, hsl], ones_sb[:], den_sb[:, hsl],
                                         start=True, stop=True)
                    db = ep.tile([128, qch], F32, tag=f"d{b}")
                    nc.vector.tensor_copy(db[:], denf[:])
                    ab = ep.tile([128, qch], F32, tag=f"a{b}")
                    nc.vector.tensor_copy(ab[:], acc[:])
                    epi.setdefault(qi, {})[b] = (db, ab)

                def emit_qi_epilogue(qi):
                    st = epi.pop(qi)
                    (d1, a1), (d2, a2) = st[0], st[1]
                    qsl = slice(qi * qch, (qi + 1) * qch)
                    r1 = ep1.tile([128, qch], F32, tag="r1")
                    r2 = ep1.tile([128, qch], F32, tag="r2")
                    nc.vector.reciprocal(r1[:], d1[:])  # = SCL / den1
                    nc.vector.reciprocal(r2[:], d2[:])
                    # m_b = attn_b_unnorm * SCL/den_b, in place over r_b
                    nc.vector.tensor_mul(r1[:], a1[:], r1[:])
                    nc.vector.tensor_mul(r2[:], a2[:], r2[:])
                    # attn' = m1 - lam*m2 (unnormalized: the per-token RMSNorm
                    # scale commutes with the out-projection and is applied at
                    # P5 PSUM eviction instead)
                    nc.vector.scalar_tensor_tensor(
                        out=attnp[:, qsl], in0=r2[:], scalar=-float(lam),
                        in1=r1[:], op0=mybir.AluOpType.mult,
                        op1=mybir.AluOpType.add,
                    )

                EPI_LAG = 4  # steps past qi end before the DVE epilogue burst
                emit_sc(0)
                for i in range(nsteps + 1 + EPI_LAG):
                    if i < nsteps:
                        emit_exp(i)
                    if i + 1 < nsteps:
                        emit_sc(i + 1)
                    if 1 <= i <= nsteps:
                        emit_denpv(i - 1)
                    j = i - 1 - EPI_LAG  # step whose qi-epilogue may be due
                    if j >= 0 and (j + 1) % (2 * nt) == 0:
                        emit_qi_epilogue(steps[j][0])

            # -------- P4+P5: rmsnorm stats + output projection --------
            # msq_raw = sum_j attn'^2 / SCL ; attn = attn'/SCL
            # R' = rsqrt(mean(attn^2) + eps)/SCL = rsqrt(msq_raw*SCL/D2 + SCL^2*eps)
            # R' is per-token: it commutes with the out-projection, so out-proj
            # runs on unnormalized attn' and R' is applied (as a per-partition
            # scalar, after a PE transpose) during PSUM eviction.
            with (
                tc.tile_pool(name="rms", bufs=2) as rmsp,
                tc.tile_pool(name="mq_ps", bufs=1, space="PSUM") as mqp,
                tc.tile_pool(name="rt_ps", bufs=2, space="PSUM") as rtp,
                tc.tile_pool(name="rt_sb", bufs=4) as rtsb,
                tc.tile_pool(name="op_ps", bufs=4, space="PSUM") as opp,
                tc.tile_pool(name="ost", bufs=3) as ostp,
            ):
                for qi in range(nq):
                    qsl = slice(qi * qch, (qi + 1) * qch)
                    # rms partial: msq = sum_j attn'^2 / SCL (ones = 1/SCL)
                    sq = rmsp.tile([128, qch], F16, tag="sq")
                    nc.vector.tensor_mul(sq[:], attnp[:, qsl], attnp[:, qsl])
                    mq = mqp.tile([128, qch], F32, tag="mq")
                    for hh in range(nhalf):
                        hsl = slice(hh * 512, (hh + 1) * 512)
                        nc.tensor.matmul(mq[:, hsl], ones_sb[:], sq[:, hsl],
                                         start=True, stop=True)
                    rs = rmsp.tile([128, qch], F32, tag="rs")
                    nc.scalar.activation(
                        rs[:], mq[:], mybir.ActivationFunctionType.Sqrt,
                        scale=float(SCL / D2), bias=eps_sb[:],
                    )
                    rr = rmsp.tile([128, qch], F32, tag="rr")
                    nc.vector.reciprocal(rr[:], rs[:])
                    for st_i in range(qi * qch // 128, (qi + 1) * qch // 128):
                        ssl = slice(st_i * 128, (st_i + 1) * 128)
                        lsl = slice((st_i * 128) % qch, (st_i * 128) % qch + 128)
                        # per-token scale to [token, 1] layout via PE transpose
                        rt_ps = rtp.tile([128, 128], F32, tag="rt")
                        nc.tensor.transpose(rt_ps[:], rr[:, lsl], ident32[:])
                        rt = rtsb.tile([128, 1], F32, tag="rts")
                        nc.vector.tensor_copy(rt[:], rt_ps[:, 0:1])
                        ot = ostp.tile([128, DIM], F16, tag="ot")
                        for hh in range(DIM // 512):
                            hsl = slice(hh * 512, (hh + 1) * 512)
                            po = opp.tile([128, 512], F32, tag="op")
                            nc.tensor.matmul(po[:], attnp[:, ssl], wo_sb[:, hsl],
                                             start=True, stop=True)
                            # eviction applies R': alternate engines
                            if hh == 0:
                                nc.vector.tensor_scalar(
                                    out=ot[:, hsl], in0=po[:], scalar1=rt[:],
                                    scalar2=None, op0=mybir.AluOpType.mult)
                            else:
                                nc.scalar.activation(
                                    ot[:, hsl], po[:],
                                    mybir.ActivationFunctionType.Copy,
                                    scale=rt[:])
                        nc.sync.dma_start(out=out[ssl, :], in_=ot[:])


def kernel(**inputs):
    x = np.asarray(inputs["x"], dtype=np.float32)          # (S, DIM)
    Wq = np.asarray(inputs["Wq"], dtype=np.float32)        # (DIM, 1024)
    Wk = np.asarray(inputs["Wk"], dtype=np.float32)
    Wv = np.asarray(inputs["Wv"], dtype=np.float32)
    Wo = np.asarray(inputs["Wo"], dtype=np.float32)        # (1024, DIM)
    lq1 = np.asarray(inputs["lambda_q1"], dtype=np.float32)
    lk1 = np.asarray(inputs["lambda_k1"], dtype=np.float32)
    lq2 = np.asarray(inputs["lambda_q2"], dtype=np.float32)
    lk2 = np.asarray(inputs["lambda_k2"], dtype=np.float32)
    subw = np.asarray(inputs["subln_weight"], dtype=np.float32)  # (128,)
    s = x.shape[0]

    lam1 = float(np.exp(np.sum(lq1 * lk1, dtype=np.float64)))
    lam2 = float(np.exp(np.sum(lq2 * lk2, dtype=np.float64)))
    lam = lam1 - lam2 + LAMBDA_INIT

    key = (s, np.float32(lam).tobytes())
    if key not in _CACHE:
        _CACHE[key] = _build(s, lam)
    nc = _CACHE[key]

    xT16 = np.ascontiguousarray(x.T).astype(np.float16)    # (DIM, S)
    scale = 1.0 / np.sqrt(np.float32(HD))
    # per-head output projection with subln weight and (1-lambda_init) folded in;
    # subw has length D2 and applies identically to every head's block of rows
    wo_f = Wo * np.tile(subw * (1.0 - LAMBDA_INIT), H)[:, None]

    def warr(w):
        # (1024, 128) -> [cl=128, ch*128+d] partition-contiguous layout
        return np.ascontiguousarray(
            w.reshape(DIM // 128, 128, D2).transpose(1, 0, 2).reshape(128, DIM // 128 * D2)
        ).astype(np.float16)

    in_maps = []
    for h in range(N_CORES):
        hsl = slice(h * D2, (h + 1) * D2)
        in_maps.append({
            "xT": xT16,
            "wq": warr(Wq[:, hsl] * scale),
            "wk": warr(Wk[:, hsl]),
            "wv": warr(Wv[:, hsl]),
            "wo": wo_f[hsl, :].astype(np.float16),
        })

    res = run_bass_kernel_spmd(nc, in_maps, list(range(N_CORES)))
    acc = np.zeros((s, DIM), dtype=np.float32)
    for i in range(N_CORES):
        acc += res.results[i]["out"].astype(np.float32)
    return acc


# revision 17
# speedup vs baseline: 1.2301x; 1.1172x over previous
"""DifferentialAttention TRN2 kernel: H=8 heads tensor-parallel across 8 NeuronCores.

Each core computes one head: both differential branches (q1k1, q2k2 softmax
attention over the shared v), the differential combine, per-head RMSNorm and
its slice of the output projection. Host sums the 8 partial outputs.

Self-contained: hardcodes shapes from the problem spec (S=4096, DIM=1024,
H=8, HD=64).
"""
import sys

sys.path.insert(0, "/opt/trn_rl_repo")

import numpy as np

import concourse.bass as bass
import concourse.mybir as mybir
import concourse.tile as tile
from concourse.bass_utils import run_bass_kernel_spmd
from concourse.masks import make_identity

S = 4096
DIM = 1024
H = 8
HD = 64
D2 = 2 * HD  # 128, per-head dim through v / rmsnorm
EPS = 1e-5
LAMBDA_INIT = 0.2
N_CORES = 8

F16 = mybir.dt.float16
F32 = mybir.dt.float32

# attn' = SCL * attn is kept scaled so fp16 intermediates stay in normal range;
# the rsqrt stage folds the 1/SCL back in exactly.
SCL = 64.0

# bench-only experiment knobs (default off; grading path never sets these)
import os as _os
K_NO_DEN = _os.environ.get("K_NO_DEN") == "1"    # skip softmax denominator adds
K_SKIP_EXP = _os.environ.get("K_SKIP_EXP") == "1"  # drop ACT exp (timing only)
K_NO_PV = _os.environ.get("K_NO_PV") == "1"      # skip pv accumulation MMs
K_SC_BUFS = int(_os.environ.get("K_SC_BUFS", "2"))
K_P_BUFS = int(_os.environ.get("K_P_BUFS", "5"))
# branch-2 exp runs on DVE (Schraudolph int16 bit trick) while branch-1 runs
# exact on ACT; the ~4% sawtooth error enters the output scaled by lambda
# (~0.35) and uniformly over each softmax row, measured ~6.6e-3 end to end.
# 0 puts all exp on ACT.
K_DVE_EXP = int(_os.environ.get("K_DVE_EXP", "1"))

# fp16 Schraudolph exp: int16 bits = round(s * 2^10*log2(e) + (15*2^10 - 59.3))
# bit-pattern == fp16(exp(s)) with |rel err| <= 4%, mean ~0 (validated on hw).
EXP_A = 1477.3197
EXP_B = 15300.7

_CACHE = {}


def _split_waits(nc, max_attached=1):
    """This container's walrus build rejects instructions carrying more than one
    attached sem wait ("Too many sync wait commands"). Hoist extras onto
    standalone EventSemaphore instructions on the same engine queue, which
    preserves semantics (per-engine program order is unchanged)."""
    for fn in nc.m.functions:
        for blk in fn.blocks:
            new = []
            for inst in blk.instructions:
                si = getattr(inst, "sync_info", None)
                if si is not None and si.on_wait is not None and len(si.on_wait) > max_attached:
                    waits = list(si.on_wait)
                    for j, w in enumerate(waits[:-max_attached]):
                        es = mybir.InstEventSemaphore(name=f"{inst.name}_hw{j}")
                        es.engine = inst.engine
                        es.sync_info = mybir.SyncInfo(on_wait=[w], on_update=[])
                        new.append(es)
                    si.on_wait = waits[-max_attached:]
                    inst.sync_info = si
                new.append(inst)
            blk.instructions = new


def _build(s, lam, reps=1):
    """Build the per-core Bass module. s = sequence length, lam = lambda_full.

    reps>1 wraps the whole body in a hardware For_i loop — used only for
    timing (wall-clock delta between rep counts isolates pure kernel time)."""
    nt = s // 128          # t tiles
    qch = min(1024, s)     # attention s-chunk ("quarter")
    nq = s // qch
    pch = min(1024, s)     # projection s-chunk
    npc = s // pch
    nc8 = DIM // 128       # contraction c-tiles

    nc = bass.Bass()
    xT = nc.declare_dram_parameter("xT", [DIM, s], F16, isOutput=False)
    # wq/wk/wv arrive pre-arranged on host as [cl=128, ch=8, d=128] (partition-
    # contiguous) so the load is one descriptor per partition instead of 1024
    # small ones
    wq = nc.declare_dram_parameter("wq", [128, DIM // 128 * D2], F16, isOutput=False)
    wk = nc.declare_dram_parameter("wk", [128, DIM // 128 * D2], F16, isOutput=False)
    wv = nc.declare_dram_parameter("wv", [128, DIM // 128 * D2], F16, isOutput=False)
    wo = nc.declare_dram_parameter("wo", [D2, DIM], F16, isOutput=False)
    out = nc.declare_dram_parameter("out", [s, DIM], F16, isOutput=True)

    from contextlib import ExitStack
    with tile.TileContext(nc) as tc:
        with (ExitStack() as _loop_ctx,):
            if reps > 1:
                _loop_ctx.enter_context(tc.For_i(0, reps, 1))
            _body(nc, tc, s, lam, nt, qch, nq, pch, npc, nc8,
                  xT, wq, wk, wv, wo, out)
    _split_waits(nc)
    return nc


def _body(nc, tc, s, lam, nt, qch, nq, pch, npc, nc8, xT, wq, wk, wv, wo, out):
        with (
            tc.tile_pool(name="singles", bufs=1) as singles,
            tc.tile_pool(name="persist", bufs=1) as persist,
        ):
            # -------- constants + weights --------
            wq_sb = singles.tile([128, nc8, D2], F16)
            wk_sb = singles.tile([128, nc8, D2], F16)
            wv_sb = singles.tile([128, nc8, D2], F16)
            nc.sync.dma_start(out=wq_sb, in_=wq[:].rearrange("cl (ch d) -> cl ch d", ch=nc8))
            nc.sync.dma_start(out=wk_sb, in_=wk[:].rearrange("cl (ch d) -> cl ch d", ch=nc8))
            nc.sync.dma_start(out=wv_sb, in_=wv[:].rearrange("cl (ch d) -> cl ch d", ch=nc8))
            wo_sb = singles.tile([128, DIM], F16)
            nc.sync.dma_start(out=wo_sb, in_=wo[:])
            ones_sb = singles.tile([128, 128], F16)
            nc.vector.memset(ones_sb, 1.0 / SCL)
            eps_sb = singles.tile([128, 1], F32)
            nc.vector.memset(eps_sb, SCL * SCL * EPS)
            ident = singles.tile([128, 128], F16)
            make_identity(nc, ident)
            ident32 = singles.tile([128, 128], F32)
            make_identity(nc, ident32)

            qT = persist.tile([128, s], F16)   # rows 0:64 branch1, 64:128 branch2
            kT = persist.tile([128, s], F16)
            v_sb = persist.tile([128, nt, 128], F16)    # v[t_tile][t_lo, d2]
            attnp = persist.tile([128, s], F16)         # SCL * (attn1 - lam*attn2), [d2, s]
            # scratch16 is vT during P1/P2, then reused as msq (sum_j attn'^2,
            # broadcast rows) — the lifetimes are disjoint
            scratch16 = persist.tile([128, s], F16)
            # xT stays resident for the whole kernel: releasing its SBUF to later
            # pools would make their first writers inherit waits on every input
            # DMA queue, overflowing the per-instruction sync-wait limit.
            xts = []
            for c in range(nc8):
                xt_c = persist.tile([128, s], F16, tag=f"xt{c}")
                xts.append(xt_c)
            vT_sb = scratch16
            msq = scratch16

            # -------- P1: q/k/v projections --------
            with (
                tc.tile_pool(name="proj_ps", bufs=3, space="PSUM") as pps,
                tc.tile_pool(name="tr_ps", bufs=2, space="PSUM") as trp,
            ):
                for c in range(nc8):
                    nc.sync.dma_start(out=xts[c], in_=xT[c * 128:(c + 1) * 128, :])
                for i in range(npc):
                    sl = slice(i * pch, (i + 1) * pch)
                    pq = pps.tile([128, pch], F32, tag="pp")
                    pk = pps.tile([128, pch], F32, tag="pp")
                    pv = pps.tile([128, pch], F32, tag="pp")
                    for c in range(nc8):
                        st_ = c == 0
                        sp_ = c == nc8 - 1
                        # group by stationary operand to minimize weight reloads
                        for w_sb, pacc in ((wq_sb, pq), (wk_sb, pk), (wv_sb, pv)):
                            for hh in range(pch // 512):
                                hsl = slice(hh * 512, (hh + 1) * 512)
                                msl = slice(i * pch + hh * 512, i * pch + (hh + 1) * 512)
                                nc.tensor.matmul(pacc[:, hsl], w_sb[:, c, :], xts[c][:, msl], start=st_, stop=sp_)
                    # alternate eviction engines so neither serializes P1
                    nc.vector.tensor_copy(qT[:, sl], pq[:])
                    nc.scalar.copy(kT[:, sl], pk[:])
                    nc.vector.tensor_copy(vT_sb[:, sl], pv[:])

                # -------- P2: transpose v to [t, d2] tiles --------
                for tt in range(nt):
                    pt = trp.tile([128, 128], F16, tag="tr")
                    nc.tensor.transpose(pt[:], vT_sb[:, tt * 128:(tt + 1) * 128], ident[:])
                    nc.vector.tensor_copy(v_sb[:, tt, :], pt[:])

            # -------- P3: attention (software-pipelined) --------
            # Steps are (qi, b, tt) flattened. Emission order per index i:
            #   exp [ACT]; sc(i+1) [PE]; den(i-1), pv(i-1) [PE]
            # so the PE always has the next step's score matmuls queued before
            # the current step's exp-dependent matmuls, and den/pv lag one step
            # to absorb branch-boundary PSUM-free waits.
            # ACT exp runs on PAIRS of steps ([128, 2*qch] super-tiles over a
            # manually-rotated 4-bank PSUM region) to amortize the scalar
            # engine's 352-cycle per-instruction overhead.
            steps = [(qi, b, tt) for qi in range(nq) for b in range(2)
                     for tt in range(nt)]
            nsteps = len(steps)
            nhalf = qch // 512
            with (
                tc.tile_pool(name="sc_ps", bufs=1, space="PSUM") as scp,
                tc.tile_pool(name="acc_ps", bufs=1, space="PSUM") as accp,
                tc.tile_pool(name="den_ps", bufs=1, space="PSUM") as denp,
                tc.tile_pool(name="ptile", bufs=K_P_BUFS) as ppool,
                tc.tile_pool(name="ep", bufs=2) as ep,
                tc.tile_pool(name="ep1", bufs=1) as ep1,
            ):
                # one [128, 2*qch] f32 super-tile = 4 PSUM banks; halves act
                # as the sc double-buffer and one exp covers both.
                scsuper = scp.tile([128, 2 * qch], F32, tag="scsuper")
                p_tiles = {}    # step idx -> (sbuf tile, offset)
                bstate = {}     # (qi, b) -> (acc, den)
                epi = {}        # per-qi: {b: (dsb, asb)}

                def emit_sc(i):
                    qi, b, tt = steps[i]
                    bsl = slice(b * 64, (b + 1) * 64)
                    tsl = slice(tt * 128, (tt + 1) * 128)
                    off = (i % 2) * qch
                    for hh in range(nhalf):
                        hsl = slice(off + hh * 512, off + (hh + 1) * 512)
                        qhs = slice(qi * qch + hh * 512, qi * qch + (hh + 1) * 512)
                        nc.tensor.matmul(scsuper[:, hsl], kT[bsl, tsl], qT[bsl, qhs],
                                         start=True, stop=True)

                def emit_exp_pair(i):
                    # one ACT instruction exps steps i and i+1 (both halves)
                    p = ppool.tile([128, 2 * qch], F16, tag="p")
                    if K_SKIP_EXP:
                        nc.gpsimd.memset(p, 1.0)  # timing-only: p off ACT path
                    else:
                        nc.scalar.activation(p[:], scsuper[:],
                                             mybir.ActivationFunctionType.Exp)
                    p_tiles[i] = (p, 0)
                    p_tiles[i + 1] = (p, qch)

                def emit_denpv(i):
                    qi, b, tt = steps[i]
                    if tt == 0:
                        acc = accp.tile([128, qch], F32, tag="acc")
                        den = denp.tile([128, qch], F32, tag="den")
                        bstate[(qi, b)] = (acc, den)
                    acc, den = bstate[(qi, b)]
                    p, off = p_tiles.pop(i)
                    st_ = tt == 0
                    sp_ = tt == nt - 1
                    if not K_NO_DEN:
                        for hh in range(nhalf):
                            hsl = slice(hh * 512, (hh + 1) * 512)
                            nc.tensor.matmul(den[:, hsl], ones_sb[:],
                                             p[:, off + hh * 512:off + (hh + 1) * 512],
                                             start=st_, stop=sp_)
                    if not K_NO_PV:
                        for hh in range(nhalf):
                            hsl = slice(hh * 512, (hh + 1) * 512)
                            nc.tensor.matmul(acc[:, hsl], v_sb[:, tt, :],
                                             p[:, off + hh * 512:off + (hh + 1) * 512],
                                             start=st_, stop=sp_)
                    if K_NO_DEN and sp_:
                        nc.tensor.matmul(den[:, 0:512], ones_sb[:], p[:, off:off + 512],
                                         start=True, stop=True)
                    if K_NO_PV and sp_:
                        nc.tensor.matmul(acc[:, 0:512], v_sb[:, 0, :], p[:, off:off + 512],
                                         start=True, stop=True)
                    if sp_:
                        emit_branch_finish(qi, b)

                def emit_branch_finish(qi, b):
                    # evacuate the single-buffered den/acc PSUM banks right
                    # away; the slow math (reciprocal etc.) is emitted a few
                    # steps later from SBUF
                    acc, den = bstate.pop((qi, b))
                    dsb = ep.tile([128, qch], F32, tag=f"d{b}")
                    nc.vector.tensor_copy(dsb[:], den[:])
                    asb = ep.tile([128, qch], F32, tag=f"a{b}")
                    nc.vector.tensor_copy(asb[:], acc[:])
                    epi.setdefault(qi, {})[b] = (dsb, asb)

                def emit_qi_epilogue(qi):
                    st = epi.pop(qi)
                    (d1, a1), (d2, a2) = st[0], st[1]
                    qsl = slice(qi * qch, (qi + 1) * qch)
                    r1 = ep1.tile([128, qch], F32, tag="r1")
                    r2 = ep1.tile([128, qch], F32, tag="r2")
                    nc.vector.reciprocal(r1[:], d1[:])  # = SCL / den1
                    nc.vector.reciprocal(r2[:], d2[:])
                    # m_b = attn_b_unnorm * SCL/den_b, in place over r_b
                    nc.vector.tensor_mul(r1[:], a1[:], r1[:])
                    nc.vector.tensor_mul(r2[:], a2[:], r2[:])
                    # attn' = m1 - lam*m2 (unnormalized: the per-token RMSNorm
                    # scale commutes with the out-projection and is applied at
                    # P5 PSUM eviction instead)
                    nc.vector.scalar_tensor_tensor(
                        out=attnp[:, qsl], in0=r2[:], scalar=-float(lam),
                        in1=r1[:], op0=mybir.AluOpType.mult,
                        op1=mybir.AluOpType.add,
                    )

                EPI_LAG = 3  # steps past qi end before the DVE epilogue burst
                emit_sc(0)
                emit_sc(1)
                for i in range(nsteps + 2 + EPI_LAG):
                    if i < nsteps and i % 2 == 0:
                        emit_exp_pair(i)
                    if i + 2 < nsteps:
                        emit_sc(i + 2)
                    if 2 <= i < nsteps + 2:
                        emit_denpv(i - 2)
                    j = i - 2 - EPI_LAG  # step whose qi-epilogue may be due
                    if j >= 0 and (j + 1) % (2 * nt) == 0:
                        emit_qi_epilogue(steps[j][0])

            # -------- P4+P5: rmsnorm stats + output projection --------
            # msq_raw = sum_j attn'^2 / SCL ; attn = attn'/SCL
            # R' = rsqrt(mean(attn^2) + eps)/SCL = rsqrt(msq_raw*SCL/D2 + SCL^2*eps)
            # R' is per-token: it commutes with the out-projection, so out-proj
            # runs on unnormalized attn' and R' is applied (as a per-partition
            # scalar, after a PE transpose) during PSUM eviction.
            with (
                tc.tile_pool(name="rms", bufs=2) as rmsp,
                tc.tile_pool(name="mq_ps", bufs=1, space="PSUM") as mqp,
                tc.tile_pool(name="rt_ps", bufs=2, space="PSUM") as rtp,
                tc.tile_pool(name="rt_sb", bufs=4) as rtsb,
                tc.tile_pool(name="op_ps", bufs=4, space="PSUM") as opp,
                tc.tile_pool(name="ost", bufs=3) as ostp,
            ):
                for qi in range(nq):
                    qsl = slice(qi * qch, (qi + 1) * qch)
                    # rms partial: msq = sum_j attn'^2 / SCL (ones = 1/SCL)
                    sq = rmsp.tile([128, qch], F16, tag="sq")
                    nc.vector.tensor_mul(sq[:], attnp[:, qsl], attnp[:, qsl])
                    mq = mqp.tile([128, qch], F32, tag="mq")
                    for hh in range(nhalf):
                        hsl = slice(hh * 512, (hh + 1) * 512)
                        nc.tensor.matmul(mq[:, hsl], ones_sb[:], sq[:, hsl],
                                         start=True, stop=True)
                    rs = rmsp.tile([128, qch], F32, tag="rs")
                    nc.scalar.activation(
                        rs[:], mq[:], mybir.ActivationFunctionType.Sqrt,
                        scale=float(SCL / D2), bias=eps_sb[:],
                    )
                    rr = rmsp.tile([128, qch], F32, tag="rr")
                    nc.vector.reciprocal(rr[:], rs[:])
                    for st_i in range(qi * qch // 128, (qi + 1) * qch // 128):
                        ssl = slice(st_i * 128, (st_i + 1) * 128)
                        lsl = slice((st_i * 128) % qch, (st_i * 128) % qch + 128)
                        # per-token scale to [token, 1] layout via PE transpose
                        rt_ps = rtp.tile([128, 128], F32, tag="rt")
                        nc.tensor.transpose(rt_ps[:], rr[:, lsl], ident32[:])
                        rt = rtsb.tile([128, 1], F32, tag="rts")
                        nc.vector.tensor_copy(rt[:], rt_ps[:, 0:1])
                        ot = ostp.tile([128, DIM], F16, tag="ot")
                        for hh in range(DIM // 512):
                            hsl = slice(hh * 512, (hh + 1) * 512)
                            po = opp.tile([128, 512], F32, tag="op")
                            nc.tensor.matmul(po[:], attnp[:, ssl], wo_sb[:, hsl],
                                             start=True, stop=True)
                            # eviction applies R': alternate engines
                            if hh == 0:
                                nc.vector.tensor_scalar(
                                    out=ot[:, hsl], in0=po[:], scalar1=rt[:],
                                    scalar2=None, op0=mybir.AluOpType.mult)
                            else:
                                nc.scalar.activation(
                                    ot[:, hsl], po[:],
                                    mybir.ActivationFunctionType.Copy,
                                    scale=rt[:])
                        nc.sync.dma_start(out=out[ssl, :], in_=ot[:])


def kernel(**inputs):
    x = np.asarray(inputs["x"], dtype=np.float32)          # (S, DIM)
    Wq = np.asarray(inputs["Wq"], dtype=np.float32)        # (DIM, 1024)
    Wk = np.asarray(inputs["Wk"], dtype=np.float32)
    Wv = np.asarray(inputs["Wv"], dtype=np.float32)
    Wo = np.asarray(inputs["Wo"], dtype=np.float32)        # (1024, DIM)
    lq1 = np.asarray(inputs["lambda_q1"], dtype=np.float32)
    lk1 = np.asarray(inputs["lambda_k1"], dtype=np.float32)
    lq2 = np.asarray(inputs["lambda_q2"], dtype=np.float32)
    lk2 = np.asarray(inputs["lambda_k2"], dtype=np.float32)
    subw = np.asarray(inputs["subln_weight"], dtype=np.float32)  # (128,)
    s = x.shape[0]

    lam1 = float(np.exp(np.sum(lq1 * lk1, dtype=np.float64)))
    lam2 = float(np.exp(np.sum(lq2 * lk2, dtype=np.float64)))
    lam = lam1 - lam2 + LAMBDA_INIT

    key = (s, np.float32(lam).tobytes())
    if key not in _CACHE:
        _CACHE[key] = _build(s, lam)
    nc = _CACHE[key]

    xT16 = np.ascontiguousarray(x.T).astype(np.float16)    # (DIM, S)
    scale = 1.0 / np.sqrt(np.float32(HD))
    # per-head output projection with subln weight and (1-lambda_init) folded in;
    # subw has length D2 and applies identically to every head's block of rows
    wo_f = Wo * np.tile(subw * (1.0 - LAMBDA_INIT), H)[:, None]

    def warr(w):
        # (1024, 128) -> [cl=128, ch*128+d] partition-contiguous layout
        return np.ascontiguousarray(
            w.reshape(DIM // 128, 128, D2).transpose(1, 0, 2).reshape(128, DIM // 128 * D2)
        ).astype(np.float16)

    in_maps = []
    for h in range(N_CORES):
        hsl = slice(h * D2, (h + 1) * D2)
        in_maps.append({
            "xT": xT16,
            "wq": warr(Wq[:, hsl] * scale),
            "wk": warr(Wk[:, hsl]),
            "wv": warr(Wv[:, hsl]),
            "wo": wo_f[hsl, :].astype(np.float16),
        })

    res = run_bass_kernel_spmd(nc, in_maps, list(range(N_CORES)))
    acc = np.zeros((s, DIM), dtype=np.float32)
    for i in range(N_CORES):
        acc += res.results[i]["out"].astype(np.float32)
    return acc
